# revision 52
# baseline (speedup 1.0000x reference)
"""Trainium2 Bass kernel for nn_BaselineOut (article/option additive-attention MRC head).

Contract: kernel(**inputs) takes FULL unsharded inputs (numpy), returns FULL
[32, 5] float32 logits.  Internally: data-parallel over batch across 8 cores
(4 batch items per core), all params replicated.

Math notes (vs reference):
  - oqc gather (question_contexts at answer_indices) is a host-side indexing
    transform; the gathered [B, H] rows ship directly in SBUF layout.
  - V-projection is pulled out of the attention sum by linearity.
  - Consecutive linear maps with no nonlinearity between are constant-folded
    on host: aq -> Qp_d via Wqv = d_Qw @ a_Vw^T, feats -> logits via
    per-option Ff_o = d_Vw^T @ f_w[:,o]^T.
  - softmax logit bias (vb) dropped (shift-invariant); exp without
    max-subtraction (|logit| small).
  - ARTICLE PATH IN FP8(e4m3) with DoubleRow matmuls (2 fp8 weights/cell =
    2x flops/cycle, measured ~215ns per K=256xN=512 matmul).  tanh output is
    fp8 so the score matmul runs DoubleRow too.  uT/Wqv also fp8+DoubleRow.
    Options K-proj stays bf16 (fp8 there fails the 2e-2 budget: only LO=32
    positions to average over, feats feed logits directly).
    Simulated end-to-end rel err ~6e-3 (tolerance 2e-2).
  - Article V-sum: DVE multiply over all h-chunks + DVE reduce for chunks
    0..5; chunks 6,7 accumulate on the ACT engine one tile later (deferred
    so they never head-block any engine FIFO).  tensor_tensor_reduce and
    gpsimd tensor ops in the steady loop measured broken/slow on HW.
  - DMA triggers cost ~650ns of issue time on the queueing engine, and the
    scalar ring shares the Activation engine with all tanh work -> weights
    ship as a few big DMAs (8 triggers), ordered wq -> wk -> wdk -> wqv so
    each lands just before its first consumer.
  - Per-partition broadcasts run on gpsimd partition_broadcast; option
    K-proj PSUM drains also go through gpsimd to keep DVE free.
  - Options K-projection hoisted into the article phase; option chains for
    (b0,b1) ride b2's article stream and b2's rides b3's, so only b3's
    option chain sits on the tail; the last article tile is split in half
    (short critical V-sum) and dummy warm-keeping matmuls bridge the PE
    through the tail drain so the GB matmuls run at K=8/8.
  - biasO = Wqv @ u computed transposed then flipped back with PE transposes.
"""

import functools
import sys

import numpy as np
import ml_dtypes

sys.path.insert(0, "/opt/trn_rl_repo")

import concourse.bass as bass  # noqa: E402
from concourse import bacc  # noqa: E402
import concourse.tile as tile  # noqa: E402
from concourse import mybir  # noqa: E402
from concourse.bass import ds, ts  # noqa: E402

B, LA, LQ, LO, H, OUT = 32, 2048, 64, 32, 1024, 5
NCORES = 8
BL = B // NCORES  # 4 batch items per core
NOPT = 5
F32 = mybir.dt.float32
BF16 = mybir.dt.bfloat16
FP8 = mybir.dt.float8e4
LT = 512  # article l-tile (free dim of the big matmuls)
NLT = LA // LT  # 4
C = H // 128  # 8 h-chunks
NPAIR = C // 2  # 4 ci-pairs for DoubleRow contraction
BO = BL * NOPT  # 20 (b, option) pairs per core
AF = mybir.ActivationFunctionType
ALU = mybir.AluOpType
AX = mybir.AxisListType
OUTP = 8  # final-linear out dim padded
BF = ml_dtypes.bfloat16
F8 = ml_dtypes.float8_e4m3
DR = mybir.MatmulPerfMode.DoubleRow
G01 = (0, 2)  # option group: b0,b1 pipelined into b2's article stream
G2 = (2, 1)  # option group: b2, pipelined into b3's article stream
GB = (3, 1)  # option group: b3 only, on the tail


def build_nc() -> bass.Bass:
    nc = bacc.Bacc("TRN2", target_bir_lowering=False, debug=False)

    # ---- DRAM I/O (per-core shard; layouts match SBUF exactly) ----
    artd = nc.dram_tensor("artT", [BL, NLT, 128, C, LT], FP8, kind="ExternalInput").ap()
    otd = nc.dram_tensor("optT", [128, C, BL, NOPT, LO], BF16, kind="ExternalInput").ap()
    oqcd = nc.dram_tensor("oqcT", [128, C, BL], FP8, kind="ExternalInput").ap()
    wQd = nc.dram_tensor("aQwT", [128, C, C, 128], FP8, kind="ExternalInput").ap()
    wKd_ = nc.dram_tensor("aKwT", [128, C, C, 128], FP8, kind="ExternalInput").ap()
    # wqv in ci-major layout for DoubleRow rhs slices
    wQVd = nc.dram_tensor("qvwT", [128, C, C, 128], FP8, kind="ExternalInput").ap()
    wDKd = nc.dram_tensor("dKwT", [128, C, C, 128], BF16, kind="ExternalInput").ap()
    vwad = nc.dram_tensor("vwaT", [128, C, 16], FP8, kind="ExternalInput").ap()
    vwdd = nc.dram_tensor("vwdT", [128, C], BF16, kind="ExternalInput").ap()
    qkbd = nc.dram_tensor("qkbT", [128, C], F32, kind="ExternalInput").ap()
    qvbd = nc.dram_tensor("qvbT", [128, C], F32, kind="ExternalInput").ap()
    fwd = nc.dram_tensor("fwT", [128, NOPT, C, OUTP], BF16, kind="ExternalInput").ap()
    fbd = nc.dram_tensor("fb", [BL, OUTP], F32, kind="ExternalInput").ap()
    id3d = nc.dram_tensor("id3", [3, 3], F32, kind="ExternalInput").ap()
    outd = nc.dram_tensor("out", [BL, OUT], F32, kind="ExternalOutput").ap()

    with (
        tile.TileContext(nc) as tc,
        nc.allow_low_precision(reason="fp8/bf16 data+weights; PE accumulates fp32"),
    ):
        with (
            tc.tile_pool(name="stream", bufs=3) as stream,
            tc.tile_pool(name="wbig", bufs=4) as wbig,
            tc.tile_pool(name="mpool", bufs=3) as mpool,
            tc.tile_pool(name="spool", bufs=2) as spool,
            tc.tile_pool(name="rpool", bufs=2) as rpool,
            tc.tile_pool(name="ubuf", bufs=2) as ubuf,
            tc.tile_pool(name="scratch", bufs=2) as scratch,
            tc.tile_pool(name="one", bufs=1) as one,
            tc.tile_pool(name="pacc", bufs=3, space="PSUM") as pacc,
            tc.tile_pool(name="prow", bufs=2, space="PSUM") as prow,
            tc.tile_pool(name="psml", bufs=2, space="PSUM") as psml,
            tc.tile_pool(name="pone", bufs=1, space="PSUM") as pone,
        ):
            # ---- DMA schedule ----
            # scalar (HW ring): qkb, wq, wk(2), wdk(2), wqv(1), fw --- in
            #   first-consumer order, few big triggers (each costs ~650ns of
            #   Activation-engine issue time).
            # sync (HW ring): article tiles; output at the end.
            # gpsimd (SW ring): small startup tiles + OT.
            qkb = one.tile([128, C], F32, tag="qkb")
            nc.scalar.dma_start(out=qkb, in_=qkbd)
            wq = wbig.tile([128, C, C, 128], FP8, tag="w")
            # wq halves split across BOTH rings so Qp can start ~10.5us
            nc.sync.dma_start(out=wq[:, 0:4], in_=wQd[:, 0:4])
            nc.scalar.dma_start(out=wq[:, 4:8], in_=wQd[:, 4:8])
            wk = wbig.tile([128, C, C, 128], FP8, tag="w")
            for co in range(0, C - 2, 2):
                nc.scalar.dma_start(out=wk[:, co : co + 2], in_=wKd_[:, co : co + 2])
            wdk = wbig.tile([128, C, C, 128], BF16, tag="w")
            nc.scalar.dma_start(out=wdk[:, 0:4], in_=wDKd[:, 0:4])
            nc.scalar.dma_start(out=wdk[:, 4:8], in_=wDKd[:, 4:8])
            wqv = wbig.tile([128, C, C, 128], FP8, tag="w")
            nc.scalar.dma_start(out=wqv, in_=wQVd)
            fw = one.tile([128, NOPT, C, OUTP], BF16, tag="fw")
            nc.scalar.dma_start(out=fw, in_=fwd)

            oqcT = one.tile([128, C, BL], FP8, tag="oqcT")
            nc.gpsimd.dma_start(out=oqcT, in_=oqcd)
            vwa8 = one.tile([128, C, 16], FP8, tag="vwa8")
            nc.gpsimd.dma_start(out=vwa8, in_=vwad)
            nc.gpsimd.dma_start(out=wk[:, C - 2 : C], in_=wKd_[:, C - 2 : C])
            vwd = one.tile([128, C], BF16, tag="vwd")
            nc.gpsimd.dma_start(out=vwd, in_=vwdd)
            qvb = one.tile([128, C], F32, tag="qvb")
            nc.gpsimd.dma_start(out=qvb, in_=qvbd)
            fb = one.tile([BL, OUTP], F32, tag="fb")
            nc.gpsimd.dma_start(out=fb, in_=fbd)
            id3 = one.tile([3, 3], F32, tag="id3")
            nc.gpsimd.dma_start(out=id3, in_=id3d)
            OT = one.tile([128, C, BL, NOPT, LO], BF16, tag="OT")
            nc.gpsimd.dma_start(out=OT, in_=otd)

            # ---------- persistent state tiles ----------
            NSLOT = NLT + 1  # last tile runs as 2 half-slots
            biasA = one.tile([128, C, BL], F32, tag="biasA")
            s_sums = one.tile([1, BL, NSLOT], F32, tag="s_sums")
            nc.vector.memset(s_sums, 0.0)
            # uT padded to stride 16 in the ci dim (DoubleRow lhsT alignment)
            uT = one.tile([128, C, 16], FP8, tag="uT")
            uTun = one.tile([128, C, BL], F32, tag="uTun")
            ssb = one.tile([1, BL], F32, tag="ssb")
            rsb = one.tile([1, BL], F32, tag="rsb")
            rs_rep = one.tile([128, BL], F32, tag="rs_rep")
            biasO = one.tile([128, C, BL], F32, tag="biasO")
            kpd_s = one.tile([128, C, BL, NOPT, LO], BF16, tag="kpd_s")
            mdt = one.tile([128, C, BL, NOPT, LO], BF16, tag="mdt")
            sboA = one.tile([3, H], F32, tag="sboA")
            sboG2 = one.tile([1, H], F32, tag="sboG2")
            sboB = one.tile([1, H], F32, tag="sboB")
            s_d = one.tile([1, BO * LO], BF16, tag="s_d")
            sdn = one.tile([1, BO * LO], BF16, tag="sdn")
            sums_d = one.tile([1, BO], F32, tag="sums_d")
            rec_d = one.tile([1, BO], F32, tag="rec_d")
            sdrep = one.tile([128, BO * LO], BF16, tag="sdrep")
            u_dT = one.tile([128, C, BO], BF16, tag="u_dT")
            poutb = pone.tile([3, 3, OUTP], F32, tag="pout")
            pout = {G01: poutb[0:2, 0], G2: poutb[0:1, 1], GB: poutb[0:1, 2]}
            sbog = {G01: sboA[0:2], G2: sboG2, GB: sboB}
            out_sA = one.tile([2, OUTP], F32, tag="out_sA")
            out_s2 = one.tile([1, OUTP], F32, tag="out_s2")
            out_sB = one.tile([1, OUTP], F32, tag="out_sB")

            # ---------- Qp / biasA (co 0-3 up front; 4-7 mid-tile0 so the
            # first kp matmuls only gate on the first weight chunks) ----------
            def qp_block(cos):
                for co in cos:
                    pq = psml.tile([128, BL], F32, tag="sml")
                    for ci in range(C):
                        nc.tensor.matmul(
                            pq,
                            lhsT=wq[:, co, ci],
                            rhs=oqcT[:, ci, :],
                            start=(ci == 0),
                            stop=(ci == C - 1),
                        )
                    nc.vector.tensor_scalar_add(
                        biasA[:, co, :], pq, qkb[:, co : co + 1]
                    )

            qp_block(range(0, 8))

            # ---------- pipelined side-work emitters (options, bf16) ----------
            def emit_kpd(co):
                kpd = pacc.tile([128, 3, NOPT, LO], F32, tag="acc")
                for ci in range(C):
                    nc.tensor.matmul(
                        kpd,
                        lhsT=wdk[:, co, ci],
                        rhs=OT[:, ci, ds(0, 3)],
                        start=(ci == 0),
                        stop=(ci == C - 1),
                    )
                nc.vector.tensor_copy(kpd_s[:, co, ds(0, 3)], kpd)

            def emit_kpd_b(co, b):
                kpd = pacc.tile([128, 1, NOPT, LO], F32, tag="acc")
                for ci in range(C):
                    nc.tensor.matmul(
                        kpd,
                        lhsT=wdk[:, co, ci],
                        rhs=OT[:, ci, ds(b, 1)],
                        start=(ci == 0),
                        stop=(ci == C - 1),
                    )
                if b == 3:
                    # DVE is on the critical uT(b3) chain at this point
                    nc.scalar.copy(kpd_s[:, co, ds(b, 1)], kpd)
                else:
                    nc.vector.tensor_copy(kpd_s[:, co, ds(b, 1)], kpd)

            # biasO for a group of n batch items, computed transposed
            # (fp8 DoubleRow over ci-pairs).
            def emit_biasO_mm(g):
                b0, n = g
                for half in range(2):
                    pbo = psml.tile([3, 512], F32, tag="sml")
                    for k in range(NPAIR):
                        nc.tensor.matmul(
                            pbo[0:n],
                            lhsT=uT[:, ds(2 * k, 2), ds(b0, n)],
                            rhs=wqv[:, ds(2 * k, 2), ds(4 * half, 4)],
                            start=(k == 0),
                            stop=(k == NPAIR - 1),
                            perf_mode=DR,
                        )
                    nc.scalar.copy(sbog[g][:, ds(half * 512, 512)], pbo[0:n])

            def emit_biasO_tr(g):
                b0, n = g
                for c in range(C):
                    ptr = psml.tile([128, 3], F32, tag="sml")
                    nc.tensor.transpose(
                        ptr[:, 0:n], sbog[g][:, ts(c, 128)], id3[0:n, 0:n]
                    )
                    nc.vector.tensor_scalar_add(
                        biasO[:, c, ds(b0, n)], ptr[:, 0:n], qvb[:, c : c + 1]
                    )

            def emit_tanh_d(g, cos):
                b0, n = g
                for co in cos:
                    for b in range(b0, b0 + n):
                        nc.scalar.activation(
                            mdt[:, co, b],
                            kpd_s[:, co, b],
                            AF.Tanh,
                            bias=biasO[:, co, b : b + 1],
                        )

            def emit_lgd(g):
                b0, n = g
                lgd = prow.tile([1, n, NOPT, LO], F32, tag="lg")
                for co in range(C):
                    nc.tensor.matmul(
                        lgd,
                        lhsT=vwd[:, co : co + 1],
                        rhs=mdt[:, co, ds(b0, n)],
                        start=(co == 0),
                        stop=(co == C - 1),
                    )
                nc.scalar.activation(
                    s_d[:, ds(b0 * NOPT * LO, n * NOPT * LO)], lgd, AF.Exp
                )

            def emit_sdn(g):
                b0, n = g
                sdv = s_d.rearrange("p (bo l) -> p bo l", l=LO)
                nc.vector.tensor_reduce(
                    sums_d[:, ds(b0 * NOPT, n * NOPT)],
                    sdv[:, ds(b0 * NOPT, n * NOPT)],
                    axis=AX.X,
                    op=ALU.add,
                )
                nc.vector.reciprocal(
                    rec_d[:, ds(b0 * NOPT, n * NOPT)],
                    sums_d[:, ds(b0 * NOPT, n * NOPT)],
                )
                nc.vector.tensor_mul(
                    sdn.rearrange("p (bo l) -> p bo l", l=LO)[
                        :, ds(b0 * NOPT, n * NOPT)
                    ],
                    sdv[:, ds(b0 * NOPT, n * NOPT)],
                    rec_d[:, ds(b0 * NOPT, n * NOPT)]
                    .unsqueeze(-1)
                    .broadcast_to((1, n * NOPT, LO)),
                )
                nc.gpsimd.partition_broadcast(
                    sdrep[:, ds(b0 * NOPT * LO, n * NOPT * LO)],
                    sdn[:, ds(b0 * NOPT * LO, n * NOPT * LO)],
                )

            def emit_opt_vsum(g):
                b0, n = g
                scr2 = scratch.tile([128, C, 3, NOPT, LO], BF16, tag="scr")
                sc = scr2[:, :, 0:n]
                nc.vector.tensor_mul(
                    sc.rearrange("p c b o l -> p c (b o l)"),
                    OT[:, :, ds(b0, n)].rearrange("p c b o l -> p c (b o l)"),
                    sdrep[:, ds(b0 * NOPT * LO, n * NOPT * LO)]
                    .unsqueeze(1)
                    .broadcast_to((128, C, n * NOPT * LO)),
                )
                nc.vector.tensor_reduce(
                    u_dT[:, :, ds(b0 * NOPT, n * NOPT)],
                    sc.rearrange("p c b o l -> p c (b o) l"),
                    axis=AX.X,
                    op=ALU.add,
                )

            def emit_final(g):
                b0, n = g
                uv = u_dT.rearrange("p c (b o) -> p c b o", o=NOPT)
                for c in range(C):
                    for o in range(NOPT):
                        nc.tensor.matmul(
                            pout[g],
                            lhsT=uv[:, c, ds(b0, n), o],
                            rhs=fw[:, o, c, :],
                            start=(c == 0 and o == 0),
                            stop=(c == C - 1 and o == NOPT - 1),
                        )

            # side-work schedule: thunks emitted after article tile (b, lt).
            side = {
                (0, 2): [lambda c=c: emit_kpd(c) for c in range(0, 2)],
                (0, 3): [lambda c=c: emit_kpd(c) for c in range(2, 4)],
                (1, 0): [lambda c=c: emit_kpd(c) for c in range(4, 6)],
                (1, 1): [lambda c=c: emit_kpd(c) for c in range(6, 8)],
                (2, 0): [lambda: emit_biasO_mm(G01)],
                (2, 1): [
                    lambda: emit_biasO_tr(G01),
                    lambda: emit_tanh_d(G01, range(0, 4)),
                ],
                (2, 2): [
                    lambda: emit_tanh_d(G01, range(4, 8)),
                    lambda: emit_lgd(G01),
                    lambda: emit_sdn(G01),
                ],
                (2, 3): [lambda: emit_opt_vsum(G01)],
                (3, 0): [
                    lambda: emit_biasO_mm(G2),
                    lambda: emit_biasO_tr(G2),
                ],
                (3, 1): [
                    lambda: emit_tanh_d(G2, range(0, 4)),
                    lambda: emit_final(G01),
                ],
                (3, 2): [
                    lambda: emit_tanh_d(G2, range(4, 8)),
                    lambda: emit_lgd(G2),
                    lambda: emit_sdn(G2),
                    lambda: emit_opt_vsum(G2),
                ],
            }

            # ---------- article branch (fp8 DoubleRow + fused V-sum) ----------
            pending = []  # deferred V-sum finishers (one tile late)
            scr_log = []  # scr tiles in emission order (for tail fillers)
            last_scrs = []

            def flush_pending():
                while pending:
                    pending.pop(0)()

            def emit_score_vsum(T, b, slot, l0, ln, all_dve, upart):
                flush_pending()
                lg = prow.tile([1, LT], F32, tag="lg")
                for half in range(NPAIR):
                    mt2 = mpool.tile([128, 2, LT], FP8, tag="mt")
                    for i in range(2):
                        co = 2 * half + i
                        kp = pacc.tile([128, LT], F32, tag="acc")
                        for k in range(NPAIR):
                            nc.tensor.matmul(
                                kp[:, 0:ln],
                                lhsT=wk[:, co, ds(2 * k, 2)],
                                rhs=T[:, ds(2 * k, 2), ds(l0, ln)],
                                start=(k == 0),
                                stop=(k == NPAIR - 1),
                                perf_mode=DR,
                            )
                        nc.scalar.activation(
                            mt2[:, i, 0:ln],
                            kp[:, 0:ln],
                            AF.Tanh,
                            bias=biasA[:, co, b : b + 1],
                        )
                    nc.tensor.matmul(
                        lg[:, 0:ln],
                        lhsT=vwa8[:, ds(2 * half, 2), 0:1],
                        rhs=mt2[:, :, 0:ln],
                        start=(half == 0),
                        stop=(half == NPAIR - 1),
                        perf_mode=DR,
                    )
                st = spool.tile([1, LT], BF16, tag="st")
                nc.scalar.activation(
                    st[:, 0:ln], lg[:, 0:ln], AF.Exp,
                    accum_out=s_sums[:, b, slot : slot + 1],
                )
                srep = rpool.tile([128, LT], BF16, tag="srep")
                nc.gpsimd.partition_broadcast(srep[:, 0:ln], st[:, 0:ln])
                scr = scratch.tile([128, C, LT], BF16, tag="scr")
                scr_log.append((scr, l0, ln))
                if all_dve:
                    # tail half-tile: pure DVE, immediate (the b3 epilogue
                    # depends on it; cross-engine splits only add coupling
                    # latency here)
                    nc.vector.tensor_mul(
                        scr[:, :, ds(l0, ln)],
                        T[:, :, ds(l0, ln)],
                        srep[:, 0:ln].unsqueeze(1).broadcast_to((128, C, ln)),
                    )
                    nc.vector.tensor_reduce(
                        upart[:, slot, :], scr[:, :, ds(l0, ln)],
                        axis=AX.X, op=ALU.add,
                    )
                else:
                    # steady tile: DVE multiplies everything (keeps the stream
                    # buffer recycle fast and gpsimd out of the chain), DVE
                    # reduces chunks 0..5; chunks 6,7 go to the ACT
                    # accumulator one tile later so they never head-block
                    # either FIFO.
                    nc.vector.tensor_mul(
                        scr[:, :, ds(l0, ln)],
                        T[:, :, ds(l0, ln)],
                        srep[:, 0:ln].unsqueeze(1).broadcast_to((128, C, ln)),
                    )
                    nc.vector.tensor_reduce(
                        upart[:, slot, 0 : C - 2], scr[:, 0 : C - 2, ds(l0, ln)],
                        axis=AX.X, op=ALU.add,
                    )

                    def finish(scr=scr, upart=upart, slot=slot, l0=l0, ln=ln):
                        sdump = scratch.tile([128, LT], BF16, tag="sdump")
                        for c in (C - 2, C - 1):
                            nc.scalar.activation(
                                sdump[:, 0:ln],
                                scr[:, c, ds(l0, ln)],
                                AF.Copy,
                                accum_out=upart[:, slot, c : c + 1],
                            )

                    pending.append(finish)

            for b in range(BL):
                upart = ubuf.tile([128, NSLOT, C], F32, tag="upart")
                nc.vector.memset(upart[:, NLT:NSLOT], 0.0)
                for lt in range(NLT):
                    T = stream.tile([128, C, LT], FP8, tag="stream")
                    nc.sync.dma_start(out=T, in_=artd[b, lt])
                    if b == BL - 1 and lt == NLT - 1:
                        # split the last tile in two halves so the tail's
                        # score -> V-sum chain covers only 256 columns
                        emit_score_vsum(T, b, lt, 0, LT // 2, False, upart)
                        emit_score_vsum(T, b, NLT, LT // 2, LT // 2, True, upart)
                        last_scrs.extend(scr_log[-2:])
                    else:
                        emit_score_vsum(T, b, lt, 0, LT, False, upart)
                    for thunk in side.get((b, lt), ()):
                        thunk()
                # per-b: normalization factor and uT (fp8 for DoubleRow biasO)
                flush_pending()
                nc.vector.tensor_reduce(
                    ssb[:, b : b + 1].unsqueeze(-1),
                    s_sums[:, b : b + 1],
                    axis=AX.X,
                    op=ALU.add,
                )
                nc.vector.reciprocal(rsb[:, b : b + 1], ssb[:, b : b + 1])
                nc.gpsimd.partition_broadcast(rs_rep[:, b : b + 1], rsb[:, b : b + 1])
                nc.vector.tensor_reduce(
                    uTun[:, :, b : b + 1],
                    upart.rearrange("p l c -> p c l"),
                    axis=AX.X,
                    op=ALU.add,
                )
                nc.vector.tensor_scalar_mul(
                    uT[:, :, b], uTun[:, :, b], rs_rep[:, b : b + 1]
                )

            # ---------- tail: b3 options + final ----------
            # final(G2) + b3's kpd fill the PE while b3's last V-sum /
            # normalization chain runs on scalar/gpsimd/DVE.
            for c in range(C):
                emit_kpd_b(c, 3)
            emit_final(G2)
            # keep-warm fillers: dummy matmuls gated on the tail halves' scr
            # so the PE stays at K=8/8 through the b3 V-sum drain and the
            # GB biasO matmuls run warm
            for sc, fl0, fln in last_scrs:
                pfl = prow.tile([1, LT], F32, tag="lg")
                for c in (0, 2, 4, 5, 7):
                    nc.tensor.matmul(
                        pfl[:, 0:fln],
                        lhsT=vwd[:, 0:1],
                        rhs=sc[:, c, ds(fl0, fln)],
                        start=True,
                        stop=True,
                    )
            emit_biasO_mm(GB)
            emit_biasO_tr(GB)
            lgd = prow.tile([1, 1, NOPT, LO], F32, tag="lg")
            for co in range(C):
                nc.scalar.activation(
                    mdt[:, co, 3],
                    kpd_s[:, co, 3],
                    AF.Tanh,
                    bias=biasO[:, co, 3:4],
                )
                nc.tensor.matmul(
                    lgd,
                    lhsT=vwd[:, co : co + 1],
                    rhs=mdt[:, co, ds(3, 1)],
                    start=(co == 0),
                    stop=(co == C - 1),
                )
            nc.scalar.activation(s_d[:, ds(3 * NOPT * LO, NOPT * LO)], lgd, AF.Exp)
            emit_sdn(GB)
            emit_opt_vsum(GB)
            emit_final(GB)
            nc.vector.tensor_add(out_sA, pout[G01], fb[0:2])
            nc.vector.tensor_add(out_s2, pout[G2], fb[0:1])
            nc.vector.tensor_add(out_sB, pout[GB], fb[0:1])
            nc.sync.dma_start(out=outd[0:2], in_=out_sA[:, 0:OUT])
            nc.sync.dma_start(out=outd[2:3], in_=out_s2[:, 0:OUT])
            nc.sync.dma_start(out=outd[3:4], in_=out_sB[:, 0:OUT])

    nc.compile()
    return nc


@functools.lru_cache(maxsize=1)
def get_nc() -> bass.Bass:
    return build_nc()


def make_in_maps(inputs: dict) -> list[dict]:
    art = np.asarray(inputs["article_contexts"], np.float32)
    qc = np.asarray(inputs["question_contexts"], np.float32)
    opt = np.asarray(inputs["options_embeds"], np.float32)
    idx = np.asarray(inputs["answer_indices"]).astype(np.int64)

    def g(name):
        return np.asarray(inputs[name], np.float32)

    def b16(x):
        return np.ascontiguousarray(np.asarray(x).astype(BF))

    def f8(x):
        return np.ascontiguousarray(np.asarray(x, np.float32).astype(F8))

    def wlay(wT):  # [H_in, H_out] -> [128, co, ci, 128] co-major chunks
        # wlay[p, co, ci, j] = wT[ci*128 + p, co*128 + j]
        return wT.reshape(C, 128, C, 128).transpose(1, 2, 0, 3)

    def wlay_ci(wT):  # ci-major: [128, ci, co, 128]
        # wlay_ci[p, ci, co, j] = wT[ci*128 + p, co*128 + j]
        return wT.reshape(C, 128, C, 128).transpose(1, 0, 2, 3)

    aQwT = f8(wlay(g("a_Qw").T))
    aKwT = f8(wlay(g("a_Kw").T))
    dKwT = b16(wlay(g("d_Kw").T))
    # folded: aq -> options query projection (ci-major for DoubleRow rhs)
    Wqv = g("d_Qw") @ g("a_Vw")  # [H, H]
    qvwT = f8(wlay_ci(Wqv.T))
    bias_qv = g("d_Qw") @ g("a_Vb") + g("d_Qb") + g("d_Kb")  # [H]
    # folded: per-option final weights
    f_w = g("f_w")  # [OUT, 5H]
    dVwT = g("d_Vw").T
    Ff = np.stack(
        [dVwT @ f_w[:, o * H : (o + 1) * H].T for o in range(NOPT)], axis=0
    )  # [o, H_in, OUT]
    fb_new = g("f_b") + sum(
        f_w[:, o * H : (o + 1) * H] @ g("d_Vb") for o in range(NOPT)
    )
    fwT = np.zeros((128, NOPT, C, OUTP), np.float32)
    fwT[:, :, :, :OUT] = Ff.reshape(NOPT, C, 128, OUT).transpose(2, 0, 1, 3)

    def colvec(v, dt):  # [H] -> [128, C] chunk-major
        a = np.ascontiguousarray(np.asarray(v, np.float32).reshape(C, 128).T)
        return np.ascontiguousarray(a.astype(dt))

    # a_vw padded to [128, C, 16] so DoubleRow lhsT slices have 16B stride
    vwaT = np.zeros((128, C, 16), np.float32)
    vwaT[:, :, 0] = colvec(g("a_vw").reshape(H), np.float32)
    vwaT = f8(vwaT)
    vwdT = colvec(g("d_vw").reshape(H), BF)
    qkbT = colvec(g("a_Qb") + g("a_Kb"), np.float32)
    qvbT = colvec(bias_qv, np.float32)

    # article: [B, NLT, 128, C, LT] so each (b, lt) tile is one contiguous
    # 4KB-per-partition DMA.  art[b, l, h] with l = lt*LT + l', h = c*128 + p.
    artT = f8(art.reshape(B, NLT, LT, C, 128).transpose(0, 1, 4, 3, 2))
    # options: [128, C, B, 5, LO]; opt[b, o, l, h] with h = c*128 + p
    optH = (
        opt.transpose(3, 0, 1, 2).reshape(C, 128, B, NOPT, LO).transpose(1, 0, 2, 3, 4)
    )
    # host-gathered question row per b, transposed: oqcT[p, c, b]
    oqc = qc[np.arange(B), idx]  # [B, H]
    oqcT = np.ascontiguousarray(oqc.reshape(B, C, 128).transpose(2, 1, 0).astype(F8))

    shared = dict(
        aQwT=aQwT, aKwT=aKwT, qvwT=qvwT, dKwT=dKwT,
        vwaT=vwaT, vwdT=vwdT, qkbT=qkbT, qvbT=qvbT,
        fwT=b16(fwT),
        fb=np.ascontiguousarray(
            np.tile(
                np.pad(fb_new.astype(np.float32), (0, 3)).reshape(1, OUTP), (BL, 1)
            )
        ),
        id3=np.eye(3, dtype=np.float32),
    )
    in_maps = []
    for r in range(NCORES):
        s = slice(r * BL, (r + 1) * BL)
        m = dict(shared)
        m["artT"] = np.ascontiguousarray(artT[s])
        m["optT"] = b16(optH[:, :, s])
        m["oqcT"] = np.ascontiguousarray(oqcT[:, :, s])
        in_maps.append(m)
    return in_maps


def run(inputs: dict, trace: bool = False, tmpdir=None):
    from concourse.bass_utils import run_bass_kernel_spmd

    nc = get_nc()
    in_maps = make_in_maps(inputs)
    res = run_bass_kernel_spmd(
        nc, in_maps, core_ids=list(range(NCORES)), trace=trace, tmpdir=tmpdir
    )
    out = np.concatenate([res.results[r]["out"] for r in range(NCORES)], axis=0)
    return np.asarray(out, np.float32), res


def kernel(**inputs) -> np.ndarray:
    out, _ = run(inputs, trace=False)
    return out
